# revision 25
# baseline (speedup 1.0000x reference)
"""AngleLoss distributed Trainium2 kernel.

mean(arccos(dot(o,t) / (|o||t|))) over 2,097,152 rows of 3-vectors,
data-parallel over 8 NeuronCores (no collective needed: each core returns
per-partition partial sums, host adds 1024 floats).

Math per row (division- and sign-free):
    dot  = sum o*t ; oo = sum o^2 ; tt = sum t^2      (bf16 compute)
    c    = dot * absrsqrt(oo*tt)                      # cos(theta)
    nump = relu(1 - c)                                # clamped 1-c
    r2   = absrsqrt(|1 - c^2|)
    g    = nump * r2        # = sqrt((1-c)/(1+c)) = tan(theta/2) in [0,inf)
    theta = 2*arctan(g)     # cayman arctan table covers [0,inf), inf->pi/2
The Arctan pass uses accum_out for the per-partition reduction.

Layout: host pre-shards rows 8 ways and stores each shard component-planar,
tile-major: tile i holds [128 partitions x (6 planes * F_i)] with each
partition's 6*F_i floats contiguous (large DMA descriptors). Both HWDGE
rings (sync + scalar) issue loads, alternating tiles.
"""

import sys

import numpy as np

if "/opt/trn_rl_repo" not in sys.path:
    sys.path.insert(0, "/opt/trn_rl_repo")

N_CORES = 8
R_TOTAL = 256 * 8192  # 2097152 rows
PER_CORE = R_TOTAL // N_CORES  # 262144
P = 128
FREE = PER_CORE // P  # 2048

import os as _os
_ts = _os.environ.get("ANGLE_TILE_SIZES")
TILE_SIZES = tuple(int(v) for v in _ts.split(",")) if _ts else (
    128, 256, 512, 640, 384, 128
)
N_INBUF = 4
assert sum(TILE_SIZES) == FREE

_BUILD_CACHE = {}


def _build_nc():
    key = (TILE_SIZES, N_INBUF)
    if key in _BUILD_CACHE:
        return _BUILD_CACHE[key]

    from concourse import bacc, mybir

    AF = mybir.ActivationFunctionType
    OP = mybir.AluOpType
    f32 = mybir.dt.float32
    bf16 = mybir.dt.bfloat16

    sizes = list(TILE_SIZES)
    T = len(sizes)
    NB = N_INBUF
    NQ = 4
    Fmax = max(sizes)
    offs = [0]
    for s in sizes:
        offs.append(offs[-1] + s)
    # cumulative value of the tile's rotating DMA sem when it completes
    tot = {}
    slot_tot = [0] * NQ
    for i in range(T):
        slot_tot[i % NQ] += 16
        tot[i] = slot_tot[i % NQ]

    nc = bacc.Bacc(
        "TRN2", target_bir_lowering=False, debug=False, num_devices=N_CORES
    )
    x = nc.dram_tensor("x", [6 * P * FREE], f32, kind="ExternalInput")
    out = nc.dram_tensor("out", [P, 32], f32, kind="ExternalOutput")
    xf = x.ap()

    def sb(name, shape, dtype):
        return nc.alloc_sbuf_tensor(name, list(shape), dtype).ap()

    inbuf = [sb(f"inb{b}", [P, 6 * Fmax], f32) for b in range(NB)]
    sqb = [sb(f"sqb{b}", [P, 6 * Fmax], bf16) for b in range(2)]
    m = sb("m", [P, 3 * Fmax], bf16)
    dxy = sb("dxy", [P, Fmax], bf16)
    dotb = [sb(f"dot{b}", [P, Fmax], bf16) for b in range(2)]
    pair = sb("pair", [P, 2 * Fmax], bf16)  # [oo1, tt1]
    oott = sb("oott", [P, 2 * Fmax], bf16)  # [oo, tt]
    prodb = [sb(f"prod{b}", [P, Fmax], bf16) for b in range(2)]
    cb = [sb(f"c{b}", [P, Fmax], bf16) for b in range(2)]
    c2v = [sb(f"c2v{b}", [P, Fmax], bf16) for b in range(2)]
    numpb = [sb(f"nump{b}", [P, Fmax], bf16) for b in range(2)]
    r1b = [sb(f"r1{b}", [P, Fmax], bf16) for b in range(2)]
    r2b = [sb(f"r2{b}", [P, Fmax], bf16) for b in range(2)]
    g_all = sb("g_all", [P, FREE], bf16)
    t_scr = sb("t_scr", [P, FREE], bf16)
    asum = sb("asum", [P, 32], f32)
    warm = sb("warm", [P, 1], bf16)
    bias0 = sb("bias0", [P, 1], f32)
    bias1 = sb("bias1", [P, 1], f32)

    S_dmaq = [nc.alloc_semaphore(f"s_dma{q}") for q in range(NQ)]
    S_dmo = nc.alloc_semaphore("s_dmo")
    S_bias = nc.alloc_semaphore("s_bias")
    S_vein = nc.alloc_semaphore("s_vein")  # 1/tile: bigmult read inputs
    S_prod = nc.alloc_semaphore("s_prod")  # 1/tile
    S_c2 = nc.alloc_semaphore("s_c2")  # 1/tile: c written
    S_veg = nc.alloc_semaphore("s_veg")  # 1/tile: g written
    S_sq = nc.alloc_semaphore("s_sq")  # 1/tile
    S_r1 = nc.alloc_semaphore("s_r1")  # 1/tile
    S_r2 = nc.alloc_semaphore("s_r2")  # 1/tile
    S_fin = nc.alloc_semaphore("s_fin")

    def dma_wait(eng, i):
        eng.wait_ge(S_dmaq[i % NQ], tot[i])

    with nc.Block(no_gpsimd_drain=True) as block:

        def issue_in_dma(eng, i):
            tile = xf[6 * P * offs[i] : 6 * P * offs[i + 1]].rearrange(
                "(p f) -> p f", p=P
            )
            eng.dma_start(
                out=inbuf[i % NB][:, : 6 * sizes[i]], in_=tile
            ).then_inc(S_dmaq[i % NQ], 16)

        def issue_guard(eng, i):
            if i >= NB:
                eng.wait_ge(S_vein, i - NB + 1)
                eng.wait_ge(S_sq, i - NB + 1)

        @block.sync
        def _(sync):
            # even tiles on sync's HWDGE ring (odd tiles go via ScalarE's)
            for i in range(0, T, 2):
                issue_guard(sync, i)
                issue_in_dma(sync, i)
            sync.wait_ge(S_fin, 1)
            sync.dma_start(out=out.ap()[:, :], in_=asum[:, :]).then_inc(
                S_dmo, 16
            )
            sync.wait_ge(S_dmo, 16)

        # Software pipeline with lag: VE iter i runs the front half of tile
        # i, then c of tile i-1, then g of tile i-2, so in steady state it
        # never waits on same-iteration ScalarE results.
        @block.vector
        def _(vector):
            vector.memset(bias0[:], 0.0).then_inc(S_bias)
            vector.memset(bias1[:], 1.0).then_inc(S_bias)
            for i in range(T + 2):
                h = i % 2
                hp = (i - 1) % 2
                hg = (i - 2) % 2
                if i < T:
                    F = sizes[i]
                    inb = inbuf[i % NB]
                    dma_wait(vector, i)
                    vector.tensor_tensor(
                        m[:, : 3 * F], inb[:, : 3 * F], inb[:, 3 * F : 6 * F],
                        OP.mult,
                    ).then_inc(S_vein)
                    vector.tensor_tensor(
                        dxy[:, :F], m[:, :F], m[:, F : 2 * F], OP.add
                    )
                    vector.tensor_tensor(
                        dotb[h][:, :F], dxy[:, :F], m[:, 2 * F : 3 * F], OP.add
                    )
                    vector.wait_ge(S_sq, i + 1)
                    sq6 = sqb[h][:, : 6 * F].rearrange("p (j f) -> p j f", j=6)
                    pr = pair[:, : 2 * F].rearrange("p (j f) -> p j f", j=2)
                    ot = oott[:, : 2 * F].rearrange("p (j f) -> p j f", j=2)
                    vector.tensor_tensor(
                        pr[:], sq6[:, 0:5:3, :], sq6[:, 1:6:3, :], OP.add
                    )
                    vector.tensor_tensor(
                        ot[:], pr[:], sq6[:, 2:6:3, :], OP.add
                    )
                    vector.tensor_tensor(
                        prodb[h][:, :F], ot[:, 0, :], ot[:, 1, :], OP.mult
                    ).then_inc(S_prod)
                if 1 <= i <= T:
                    F = sizes[i - 1]
                    vector.wait_ge(S_r1, i)
                    vector.tensor_tensor(
                        cb[hp][:, :F], dotb[hp][:, :F], r1b[hp][:, :F],
                        OP.mult,
                    )
                    vector.tensor_tensor(
                        c2v[hp][:, :F], cb[hp][:, :F], cb[hp][:, :F], OP.mult
                    ).then_inc(S_c2)
                if i >= 2:
                    F = sizes[i - 2]
                    vector.wait_ge(S_r2, i - 1)
                    vector.tensor_tensor(
                        g_all[:, offs[i - 2] : offs[i - 1]],
                        numpb[hg][:, :F], r2b[hg][:, :F], OP.mult,
                    ).then_inc(S_veg)

        @block.scalar
        def _(scalar):
            def triple(i):
                # nump/r2 for tile i (reads cb/c2v written by VE)
                hh = i % 2
                F = sizes[i]
                scalar.wait_ge(S_c2, i + 1)
                scalar.activation(
                    numpb[hh][:, :F], cb[hh][:, :F], AF.Relu,
                    bias=bias1[:], scale=-1.0,
                )
                scalar.activation(
                    r2b[hh][:, :F], c2v[hh][:, :F], AF.Abs_reciprocal_sqrt,
                    bias=bias1[:], scale=-1.0,
                ).then_inc(S_r2)

            # odd ramp tiles are issued on ScalarE's HWDGE ring
            for j in range(1, min(NB, T), 2):
                issue_in_dma(scalar, j)
            # first activation in program order pins the absrsqrt table set;
            # bias=warm itself avoids needing an initialized constant
            scalar.activation(
                warm[:], warm[:], AF.Abs_reciprocal_sqrt, bias=warm[:],
                scale=0.0,
            )
            scalar.wait_ge(S_bias, 2)
            dma_wait(scalar, 0)
            scalar.activation(
                sqb[0][:, : 6 * sizes[0]], inbuf[0][:, : 6 * sizes[0]],
                AF.Square, bias=bias0[:],
            ).then_inc(S_sq)
            for i in range(T):
                h = i % 2
                if i + 1 < T:
                    # sq[i+1] ahead of r1[i] so VE's pair-adds for tile i+1
                    # are never starved behind this iteration's r1/r2
                    hn = (i + 1) % 2
                    F1 = sizes[i + 1]
                    dma_wait(scalar, i + 1)
                    if i + 1 >= 2:
                        # sqb[hn] free: tile i-1's pair-adds are done
                        scalar.wait_ge(S_prod, i)
                    scalar.activation(
                        sqb[hn][:, : 6 * F1],
                        inbuf[(i + 1) % NB][:, : 6 * F1],
                        AF.Square, bias=bias0[:],
                    ).then_inc(S_sq)
                F = sizes[i]
                scalar.wait_ge(S_prod, i + 1)
                scalar.activation(
                    r1b[h][:, :F], prodb[h][:, :F], AF.Abs_reciprocal_sqrt,
                    bias=bias0[:],
                ).then_inc(S_r1)
                if i + NB < T and (i + NB) % 2 == 1:
                    # inbuf[(i+NB)%NB] free: implied by S_prod>=i+1 (VE's
                    # bigmult of tile i) + own sq[i] earlier
                    issue_in_dma(scalar, i + NB)
                if i >= 1:
                    triple(i - 1)
            triple(T - 1)
            # dummy arctan: forces the sigmoid-set table load now,
            # overlapping VE's final g multiplies
            scalar.activation(
                warm[:], warm[:], AF.Arctan, bias=bias0[:], scale=0.0
            )
            scalar.wait_ge(S_veg, T)
            scalar.activation(
                t_scr[:], g_all[:], AF.Arctan, bias=bias0[:],
                accum_out=asum[:, 0:1],
            ).then_inc(S_fin)

    nc.compile()
    _BUILD_CACHE[key] = nc
    return nc


def _shard_inputs(outputs, targets):
    o = np.ascontiguousarray(np.asarray(outputs), dtype=np.float32).reshape(-1, 3)
    t = np.ascontiguousarray(np.asarray(targets), dtype=np.float32).reshape(-1, 3)
    in_maps = []
    for cidx in range(N_CORES):
        lo, hi = cidx * PER_CORE, (cidx + 1) * PER_CORE
        oc = o[lo:hi]
        tc_ = t[lo:hi]
        planes = np.empty((6, P, FREE), dtype=np.float32)
        for k in range(3):
            planes[k] = oc[:, k].reshape(P, FREE)
            planes[3 + k] = tc_[:, k].reshape(P, FREE)
        # tile-major flat: per tile, [P, 6, F_i] with rows contiguous
        blocks = []
        off = 0
        for F in TILE_SIZES:
            blk = planes[:, :, off : off + F]  # [6, P, F]
            blocks.append(
                np.ascontiguousarray(blk.transpose(1, 0, 2)).reshape(-1)
            )
            off += F
        in_maps.append({"x": np.concatenate(blocks)})
    return in_maps


LAST_RESULT = None


def kernel(outputs, targets):
    global LAST_RESULT
    import os

    from concourse.bass_utils import run_bass_kernel_spmd

    nc = _build_nc()
    in_maps = _shard_inputs(outputs, targets)
    trace = bool(os.environ.get("ANGLE_KERNEL_TRACE"))
    res = run_bass_kernel_spmd(
        nc, in_maps, core_ids=list(range(N_CORES)), trace=trace
    )
    LAST_RESULT = res
    total = 0.0
    for rmap in res.results:
        total += np.asarray(rmap["out"], dtype=np.float64)[:, 0].sum()
    mean = 2.0 * total / R_TOTAL
    return np.float32(mean)


# revision 26
# speedup vs baseline: 1.2373x; 1.2373x over previous
"""AngleLoss distributed Trainium2 kernel.

mean(arccos(dot(o,t) / (|o||t|))) over 2,097,152 rows of 3-vectors,
data-parallel over 8 NeuronCores (no collective needed: each core returns
per-partition partial sums, host adds 1024 floats).

Math per row (division- and sign-free):
    dot  = sum o*t ; oo = sum o^2 ; tt = sum t^2      (bf16 compute)
    c    = dot * absrsqrt(oo*tt)                      # cos(theta)
    nump = relu(1 - c)                                # clamped 1-c
    r2   = absrsqrt(|1 - c^2|)
    g    = nump * r2        # = sqrt((1-c)/(1+c)) = tan(theta/2) in [0,inf)
    theta = 2*arctan(g)     # cayman arctan table covers [0,inf), inf->pi/2
The Arctan pass uses accum_out for the per-partition reduction.

Layout: host pre-shards rows 8 ways and stores each shard component-planar,
tile-major: tile i holds [128 partitions x (6 planes * F_i)] with each
partition's 6*F_i floats contiguous (large DMA descriptors). Both HWDGE
rings (sync + scalar) issue loads, alternating tiles.
"""

import sys

import numpy as np

if "/opt/trn_rl_repo" not in sys.path:
    sys.path.insert(0, "/opt/trn_rl_repo")

N_CORES = 8
R_TOTAL = 256 * 8192  # 2097152 rows
PER_CORE = R_TOTAL // N_CORES  # 262144
P = 128
FREE = PER_CORE // P  # 2048

import os as _os
_ts = _os.environ.get("ANGLE_TILE_SIZES")
TILE_SIZES = tuple(int(v) for v in _ts.split(",")) if _ts else (
    64, 128, 192, 256, 384, 384, 320, 192, 128
)
N_INBUF = 4
assert sum(TILE_SIZES) == FREE

_BUILD_CACHE = {}


def _build_nc():
    key = (TILE_SIZES, N_INBUF)
    if key in _BUILD_CACHE:
        return _BUILD_CACHE[key]

    from concourse import bacc, mybir

    AF = mybir.ActivationFunctionType
    OP = mybir.AluOpType
    f32 = mybir.dt.float32
    bf16 = mybir.dt.bfloat16

    sizes = list(TILE_SIZES)
    T = len(sizes)
    NB = N_INBUF
    NQ = 4
    Fmax = max(sizes)
    offs = [0]
    for s in sizes:
        offs.append(offs[-1] + s)
    # cumulative value of the tile's rotating DMA sem when it completes
    tot = {}
    slot_tot = [0] * NQ
    for i in range(T):
        slot_tot[i % NQ] += 16
        tot[i] = slot_tot[i % NQ]

    nc = bacc.Bacc(
        "TRN2", target_bir_lowering=False, debug=False, num_devices=N_CORES
    )
    x = nc.dram_tensor("x", [6 * P * FREE], f32, kind="ExternalInput")
    out = nc.dram_tensor("out", [P, 32], f32, kind="ExternalOutput")
    xf = x.ap()

    def sb(name, shape, dtype):
        return nc.alloc_sbuf_tensor(name, list(shape), dtype).ap()

    inbuf = [sb(f"inb{b}", [P, 6 * Fmax], f32) for b in range(NB)]
    sqb = [sb(f"sqb{b}", [P, 6 * Fmax], bf16) for b in range(2)]
    m = sb("m", [P, 3 * Fmax], bf16)
    dxy = sb("dxy", [P, Fmax], bf16)
    dotb = [sb(f"dot{b}", [P, Fmax], bf16) for b in range(2)]
    pair = sb("pair", [P, 2 * Fmax], bf16)  # [oo1, tt1]
    oott = sb("oott", [P, 2 * Fmax], bf16)  # [oo, tt]
    prodb = [sb(f"prod{b}", [P, Fmax], bf16) for b in range(2)]
    cb = [sb(f"c{b}", [P, Fmax], bf16) for b in range(2)]
    c2v = [sb(f"c2v{b}", [P, Fmax], bf16) for b in range(2)]
    numpb = [sb(f"nump{b}", [P, Fmax], bf16) for b in range(2)]
    r1b = [sb(f"r1{b}", [P, Fmax], bf16) for b in range(2)]
    r2b = [sb(f"r2{b}", [P, Fmax], bf16) for b in range(2)]
    g_all = sb("g_all", [P, FREE], bf16)
    t_scr = sb("t_scr", [P, FREE], bf16)
    asum = sb("asum", [P, 32], f32)
    warm = sb("warm", [P, 1], bf16)
    bias0 = sb("bias0", [P, 1], f32)
    bias1 = sb("bias1", [P, 1], f32)

    S_dmaq = [nc.alloc_semaphore(f"s_dma{q}") for q in range(NQ)]
    S_dmo = nc.alloc_semaphore("s_dmo")
    S_bias = nc.alloc_semaphore("s_bias")
    S_vein = nc.alloc_semaphore("s_vein")  # 1/tile: bigmult read inputs
    S_prod = nc.alloc_semaphore("s_prod")  # 1/tile
    S_c2 = nc.alloc_semaphore("s_c2")  # 1/tile: c written
    S_veg = nc.alloc_semaphore("s_veg")  # 1/tile: g written
    S_sq = nc.alloc_semaphore("s_sq")  # 1/tile
    S_r1 = nc.alloc_semaphore("s_r1")  # 1/tile
    S_r2 = nc.alloc_semaphore("s_r2")  # 1/tile
    S_fin = nc.alloc_semaphore("s_fin")

    def dma_wait(eng, i):
        eng.wait_ge(S_dmaq[i % NQ], tot[i])

    with nc.Block(no_gpsimd_drain=True) as block:

        def issue_in_dma(eng, i):
            tile = xf[6 * P * offs[i] : 6 * P * offs[i + 1]].rearrange(
                "(p f) -> p f", p=P
            )
            eng.dma_start(
                out=inbuf[i % NB][:, : 6 * sizes[i]], in_=tile
            ).then_inc(S_dmaq[i % NQ], 16)

        def issue_guard(eng, i):
            if i >= NB:
                eng.wait_ge(S_vein, i - NB + 1)
                eng.wait_ge(S_sq, i - NB + 1)

        @block.sync
        def _(sync):
            # even tiles on sync's HWDGE ring (odd tiles go via ScalarE's)
            for i in range(0, T, 2):
                issue_guard(sync, i)
                issue_in_dma(sync, i)
            sync.wait_ge(S_fin, 1)
            sync.dma_start(out=out.ap()[:, :], in_=asum[:, :]).then_inc(
                S_dmo, 16
            )
            sync.wait_ge(S_dmo, 16)

        # Software pipeline with lag: VE iter i runs the front half of tile
        # i, then c of tile i-1, then g of tile i-2, so in steady state it
        # never waits on same-iteration ScalarE results.
        @block.vector
        def _(vector):
            vector.memset(bias0[:], 0.0).then_inc(S_bias)
            vector.memset(bias1[:], 1.0).then_inc(S_bias)
            for i in range(T + 2):
                h = i % 2
                hp = (i - 1) % 2
                hg = (i - 2) % 2
                if i < T:
                    F = sizes[i]
                    inb = inbuf[i % NB]
                    dma_wait(vector, i)
                    vector.tensor_tensor(
                        m[:, : 3 * F], inb[:, : 3 * F], inb[:, 3 * F : 6 * F],
                        OP.mult,
                    ).then_inc(S_vein)
                    vector.tensor_tensor(
                        dxy[:, :F], m[:, :F], m[:, F : 2 * F], OP.add
                    )
                    vector.tensor_tensor(
                        dotb[h][:, :F], dxy[:, :F], m[:, 2 * F : 3 * F], OP.add
                    )
                    vector.wait_ge(S_sq, i + 1)
                    sq6 = sqb[h][:, : 6 * F].rearrange("p (j f) -> p j f", j=6)
                    pr = pair[:, : 2 * F].rearrange("p (j f) -> p j f", j=2)
                    ot = oott[:, : 2 * F].rearrange("p (j f) -> p j f", j=2)
                    vector.tensor_tensor(
                        pr[:], sq6[:, 0:5:3, :], sq6[:, 1:6:3, :], OP.add
                    )
                    vector.tensor_tensor(
                        ot[:], pr[:], sq6[:, 2:6:3, :], OP.add
                    )
                    vector.tensor_tensor(
                        prodb[h][:, :F], ot[:, 0, :], ot[:, 1, :], OP.mult
                    ).then_inc(S_prod)
                if 1 <= i <= T:
                    F = sizes[i - 1]
                    vector.wait_ge(S_r1, i)
                    vector.tensor_tensor(
                        cb[hp][:, :F], dotb[hp][:, :F], r1b[hp][:, :F],
                        OP.mult,
                    )
                    vector.tensor_tensor(
                        c2v[hp][:, :F], cb[hp][:, :F], cb[hp][:, :F], OP.mult
                    ).then_inc(S_c2)
                if i >= 2:
                    F = sizes[i - 2]
                    vector.wait_ge(S_r2, i - 1)
                    vector.tensor_tensor(
                        g_all[:, offs[i - 2] : offs[i - 1]],
                        numpb[hg][:, :F], r2b[hg][:, :F], OP.mult,
                    ).then_inc(S_veg)

        @block.scalar
        def _(scalar):
            def triple(i):
                # nump/r2 for tile i (reads cb/c2v written by VE)
                hh = i % 2
                F = sizes[i]
                scalar.wait_ge(S_c2, i + 1)
                scalar.activation(
                    numpb[hh][:, :F], cb[hh][:, :F], AF.Relu,
                    bias=bias1[:], scale=-1.0,
                )
                scalar.activation(
                    r2b[hh][:, :F], c2v[hh][:, :F], AF.Abs_reciprocal_sqrt,
                    bias=bias1[:], scale=-1.0,
                ).then_inc(S_r2)

            # odd ramp tiles are issued on ScalarE's HWDGE ring
            for j in range(1, min(NB, T), 2):
                issue_in_dma(scalar, j)
            # first activation in program order pins the absrsqrt table set;
            # bias=warm itself avoids needing an initialized constant
            scalar.activation(
                warm[:], warm[:], AF.Abs_reciprocal_sqrt, bias=warm[:],
                scale=0.0,
            )
            scalar.wait_ge(S_bias, 2)
            dma_wait(scalar, 0)
            scalar.activation(
                sqb[0][:, : 6 * sizes[0]], inbuf[0][:, : 6 * sizes[0]],
                AF.Square, bias=bias0[:],
            ).then_inc(S_sq)
            for i in range(T):
                h = i % 2
                if i + 1 < T:
                    # sq[i+1] ahead of r1[i] so VE's pair-adds for tile i+1
                    # are never starved behind this iteration's r1/r2
                    hn = (i + 1) % 2
                    F1 = sizes[i + 1]
                    dma_wait(scalar, i + 1)
                    if i + 1 >= 2:
                        # sqb[hn] free: tile i-1's pair-adds are done
                        scalar.wait_ge(S_prod, i)
                    scalar.activation(
                        sqb[hn][:, : 6 * F1],
                        inbuf[(i + 1) % NB][:, : 6 * F1],
                        AF.Square, bias=bias0[:],
                    ).then_inc(S_sq)
                F = sizes[i]
                scalar.wait_ge(S_prod, i + 1)
                scalar.activation(
                    r1b[h][:, :F], prodb[h][:, :F], AF.Abs_reciprocal_sqrt,
                    bias=bias0[:],
                ).then_inc(S_r1)
                if i + NB < T and (i + NB) % 2 == 1:
                    # inbuf[(i+NB)%NB] free: implied by S_prod>=i+1 (VE's
                    # bigmult of tile i) + own sq[i] earlier
                    issue_in_dma(scalar, i + NB)
                if i >= 1:
                    triple(i - 1)
            triple(T - 1)
            # dummy arctan: forces the sigmoid-set table load now,
            # overlapping VE's final g multiplies
            scalar.activation(
                warm[:], warm[:], AF.Arctan, bias=bias0[:], scale=0.0
            )
            scalar.wait_ge(S_veg, T)
            scalar.activation(
                t_scr[:], g_all[:], AF.Arctan, bias=bias0[:],
                accum_out=asum[:, 0:1],
            ).then_inc(S_fin)

    nc.compile()
    _BUILD_CACHE[key] = nc
    return nc


def _shard_inputs(outputs, targets):
    o = np.ascontiguousarray(np.asarray(outputs), dtype=np.float32).reshape(-1, 3)
    t = np.ascontiguousarray(np.asarray(targets), dtype=np.float32).reshape(-1, 3)
    in_maps = []
    for cidx in range(N_CORES):
        lo, hi = cidx * PER_CORE, (cidx + 1) * PER_CORE
        oc = o[lo:hi]
        tc_ = t[lo:hi]
        planes = np.empty((6, P, FREE), dtype=np.float32)
        for k in range(3):
            planes[k] = oc[:, k].reshape(P, FREE)
            planes[3 + k] = tc_[:, k].reshape(P, FREE)
        # tile-major flat: per tile, [P, 6, F_i] with rows contiguous
        blocks = []
        off = 0
        for F in TILE_SIZES:
            blk = planes[:, :, off : off + F]  # [6, P, F]
            blocks.append(
                np.ascontiguousarray(blk.transpose(1, 0, 2)).reshape(-1)
            )
            off += F
        in_maps.append({"x": np.concatenate(blocks)})
    return in_maps


LAST_RESULT = None


def kernel(outputs, targets):
    global LAST_RESULT
    import os

    from concourse.bass_utils import run_bass_kernel_spmd

    nc = _build_nc()
    in_maps = _shard_inputs(outputs, targets)
    trace = bool(os.environ.get("ANGLE_KERNEL_TRACE"))
    res = run_bass_kernel_spmd(
        nc, in_maps, core_ids=list(range(N_CORES)), trace=trace
    )
    LAST_RESULT = res
    total = 0.0
    for rmap in res.results:
        total += np.asarray(rmap["out"], dtype=np.float64)[:, 0].sum()
    mean = 2.0 * total / R_TOTAL
    return np.float32(mean)
